# revision 25
# baseline (speedup 1.0000x reference)
"""Trainium2 Bass kernel for nn_AttentionModule_53223234732422.

Computes: RMSNorm -> QKV projections -> interleaved-pair RoPE on Q,K ->
causal softmax attention (16 heads, head_dim 128) -> output projection.

Sharding (8 NeuronCores, tensor parallel over heads):
  - every core computes the RMSNorm (cheap, avoids an activation collective),
  - each core owns 2 heads: QKV projections with column-sliced weights,
    RoPE, causal attention for those heads,
  - per-head context is AllGathered in bf16 (2 x 0.5 MiB per rank); the
    output projection runs bf16 x bf16 (wo converted on device),
  - output projection is split column-wise: each core produces 256 output
    features from the full gathered context.

Host-side preparation (layout only):
  - ALL inputs are packed into a single flat f32r dram blob per core
    (xsT | wqT | wkT | wvT | woT | cos | sin | tri): each extra PJRT
    operand costs measurable per-execute marshaling time under axon,
  - xs transposed to feature-major [E, S] so contractions land on SBUF
    partitions,
  - norm_w folded into the QKV weights,
  - wq/wk rows permuted per head so RoPE pairs are deinterleaved
    (x0 rows 0..63, x1 rows 64..127); scores are permutation invariant,
  - weights pre-rounded to fp32r (11 mantissa bits, RNE) to match the
    on-device rounding path,
  - cos/sin tables (fp16-arange thetas, like the reference) and the 4
    diagonal causal-mask tiles precomputed.

Dtypes: all matmuls run fp32r (full-rate fp32 path on the PE, 11 mantissa
bits, ~1.2e-4 input rounding; plain fp32 would be 4x slower). PSUM
accumulation is fp32 everywhere. Every tensor an fp32r matmul consumes is
written only by fp32r-typed producers (BIR verifier requirement); weights
are pre-rounded on the host and DMA'd with fp32r-typed endpoints.
Measured on HW: end-to-end relative error 2.4e-3 vs the fp32 reference
(fp32r matmul rounding ~2e-4 + bf16 context/wo ~2e-3; gate is 2e-2).
"""

import sys

sys.path.insert(0, "/opt/trn_rl_repo")

import numpy as np

import concourse.bacc as bacc
import concourse.mybir as mybir
import concourse.tile as tile
from concourse.bass import ds, ts

dt = mybir.dt
AF = mybir.ActivationFunctionType
ALU = mybir.AluOpType

S = 2048
E = 2048
H = 16
D = 128
HALF = D // 2
EPS = 1e-6
THETA = 10000.0
N_CORES = 8
HPC = H // N_CORES  # heads per core
JC = HPC * D  # 256: local q/k/v width
EB = E // N_CORES  # 256: output columns per core
ET = E // 128  # 16 feature tiles
TT = S // 128  # 16 token tiles
NS = S // 512  # 4 token strips
CH = ET // 2  # 8 e-tiles per contraction chunk
INV_SQRT_D = float(1.0 / np.sqrt(np.float32(D)))

F32 = dt.float32
F32R = dt.float32r
BF16 = dt.bfloat16

# Single packed input blob (element offsets, f32/f32r are both 4 bytes).
# One ExternalInput instead of eight: each extra operand costs real
# per-execute marshaling time in the axon PJRT dispatch path.
NE_XS = E * S
NE_W = E * JC  # == E * EB for woT
NE_TRIG = D * S
NE_TRI = 128 * 128
OFF_XS = 0
OFF_WQ = OFF_XS + NE_XS
OFF_WK = OFF_WQ + NE_W
OFF_WV = OFF_WK + NE_W
OFF_WO = OFF_WV + NE_W
OFF_COS = OFF_WO + NE_W
OFF_SIN = OFF_COS + NE_TRIG
OFF_TRI = OFF_SIN + NE_TRIG
NE_TOT = OFF_TRI + NE_TRI

_NC_CACHE = {}


def _build_nc():
    nc = bacc.Bacc(trn_type="TRN2", num_devices=N_CORES)

    blob = nc.dram_tensor("blob", [NE_TOT], F32R, kind="ExternalInput")
    out_ext = nc.dram_tensor("out", [S, EB], F32, kind="ExternalOutput")

    def xs_rows(e):
        return blob[ds(OFF_XS + e * 128 * S, 128 * S)].rearrange(
            "(p t) -> p t", p=128
        )

    def w_chunk(base, chunk):
        return blob[ds(base + chunk * CH * 128 * JC, CH * 128 * JC)].rearrange(
            "(a p j) -> p a j", p=128, j=JC
        )

    def trig(base):
        return (
            blob[ds(base, NE_TRIG)]
            .bitcast(F32)
            .rearrange("(p t) -> p t", p=128)
        )

    rg = [list(range(N_CORES))]

    with tile.TileContext(nc) as tc:
        with (
            tc.tile_pool(name="persist", bufs=1) as pp,
            tc.tile_pool(name="dram", bufs=1, space="DRAM") as dpool,
        ):
            ones_f = pp.tile([128, 1], F32, tag="ones_f")
            ones_r = pp.tile([128, 1], F32R, tag="ones_r")
            ones_b = pp.tile([128, 1], BF16, tag="ones_b")
            eps_sc = pp.tile([1, 1], F32, tag="eps_sc")
            nc.vector.memset(ones_f[:], 1.0)
            nc.vector.tensor_copy(ones_r[:], ones_f[:])
            nc.vector.tensor_copy(ones_b[:], ones_f[:])
            nc.vector.memset(eps_sc[:], EPS)

            # RoPE'd q/k (fp32r; written only by the final rope add) and
            # bf16 token-major v.
            qrope = pp.tile([128, HPC * S], F32R, tag="qrope")
            krope = pp.tile([128, HPC * S], F32R, tag="krope")
            v_sb = pp.tile([128, TT * JC], F32R, tag="v_sb")

            # context is gathered in bf16: halves the AllGather payload and
            # the Phase-E HBM reads; output error stays ~2-4e-3 (gate 2e-2)
            cbounce = [
                dpool.tile([128, S], BF16, tag=f"cb{m}", name=f"cb{m}")
                for m in range(HPC)
            ]
            ag_out = [
                dpool.tile(
                    [N_CORES * 128, S],
                    BF16,
                    addr_space="Shared",
                    tag=f"ag{m}",
                    name=f"ag{m}",
                )
                for m in range(HPC)
            ]

            # ---------- Phase A+C: fused RMS + QKV, xs read ONCE -----------
            # The 1/rms per-token scale commutes out of the e-contraction, so
            # QKV consumes RAW xs (host-pre-rounded to fp32r); the scale is
            # folded into the cos/sin tables (q/k) and a v post-pass.
            # Pass 1 streams e-tiles 8..15, squares them for the running ssq
            # AND contracts them for QKV while resident; pass 2 does the same
            # for tiles 0..7, then computes rms, folds it into the rope
            # tables, ropes q/k, and scales v. This avoids the 8 MiB chunk-1
            # re-read the 2-phase version paid.
            with tc.tile_pool(name="bcC", bufs=1) as bcp:
                bcastR = bcp.tile([128, S], F32, tag="bcastR")
                recip_col = bcp.tile([128, TT], F32, tag="recip_col")
                with (
                    tc.tile_pool(name="xsp", bufs=CH) as xsp,
                    tc.tile_pool(name="rmsp", bufs=1) as rmsp,
                    tc.tile_pool(name="sqp", bufs=2) as sqp,
                    tc.tile_pool(name="wch", bufs=3) as wchp,
                    tc.tile_pool(name="acc", bufs=1) as accp,
                    tc.tile_pool(name="trig", bufs=1) as trigp,
                    tc.tile_pool(name="rsw", bufs=1) as rsp,
                    tc.tile_pool(name="psA", bufs=NS, space="PSUM") as psA,
                    tc.tile_pool(name="psQK", bufs=2, space="PSUM") as psQK,
                    tc.tile_pool(name="psV", bufs=2, space="PSUM") as psV,
                ):
                    rms_row = rmsp.tile([1, S], F32, tag="rms_row")
                    ssq_ps = [
                        psA.tile([1, 512], F32, tag="ssq", name="ssq")
                        for _ in range(NS)
                    ]
                    cos_sb = trigp.tile([D, S], F32, tag="cos_sb")
                    sin_sb = trigp.tile([D, S], F32, tag="sin_sb")
                    nc.sync.dma_start(cos_sb[:], trig(OFF_COS))
                    nc.sync.dma_start(sin_sb[:], trig(OFF_SIN))
                    qacc = accp.tile([128, HPC * S], F32, tag="qacc")
                    kacc = accp.tile([128, HPC * S], F32, tag="kacc")

                    def load_weights(chunk):
                        wtiles = []
                        for wname, wbase in (
                            ("wq", OFF_WQ),
                            ("wk", OFF_WK),
                            ("wv", OFF_WV),
                        ):
                            wc = wchp.tile(
                                [128, CH * JC], F32R, tag="wch",
                                name=f"w{chunk}_{wname}",
                            )
                            nc.sync.dma_start(
                                wc[:].rearrange("p (a j) -> p a j", a=CH),
                                w_chunk(wbase, chunk),
                            )
                            wtiles.append(wc)
                        return wtiles

                    def stream_xs(chunk, first):
                        xh = []
                        for i in range(CH):
                            e = chunk * CH + i
                            xt = xsp.tile([128, S], F32R, tag="xsA", name="xsA")
                            nc.sync.dma_start(xt[:], xs_rows(e))
                            xh.append(xt)
                            for s in range(NS):
                                sq = sqp.tile(
                                    [128, 512], F32R, tag="sq", name="sq"
                                )
                                nc.vector.tensor_mul(
                                    sq[:],
                                    xt[:, ts(s, 512)].bitcast(F32),
                                    xt[:, ts(s, 512)].bitcast(F32),
                                )
                                nc.tensor.matmul(
                                    ssq_ps[s][:],
                                    ones_r[:],
                                    sq[:],
                                    start=(first and i == 0),
                                    stop=((not first) and i == CH - 1),
                                )
                        return xh

                    def qkv_chunk(xh, wq_c, wk_c, wv_c, first):
                        # v projection FIRST -> token-major [t, j], raw sum
                        # (the 1/rms scale is applied in a post-pass):
                        # attention consumes v, so finishing it early lets
                        # head-0 attention start under the q/k tail.
                        for t in range(TT):
                            ps = psV.tile([128, JC], F32, tag="v_ps", name="v_ps")
                            for i in range(CH):
                                nc.tensor.matmul(
                                    ps[:],
                                    xh[i][:, ts(t, 128)],
                                    wv_c[:, ts(i, JC)],
                                    start=(i == 0),
                                    stop=(i == CH - 1),
                                )
                            vsl = v_sb[:, ts(t, JC)]
                            if first:
                                nc.vector.tensor_copy(vsl, ps[:])
                            else:
                                nc.vector.scalar_tensor_tensor(
                                    vsl,
                                    ps[:],
                                    1.0,
                                    vsl.bitcast(F32),
                                    ALU.mult,
                                    ALU.add,
                                )
                        if not first:
                            # apply the per-token 1/rms right away so v is
                            # attention-ready before the rope DVE work queues
                            for t in range(TT):
                                vsl = v_sb[:, ts(t, JC)]
                                nc.vector.tensor_scalar_mul(
                                    vsl, vsl.bitcast(F32), recip_col[:, t : t + 1]
                                )

                        # q and k projections -> d-major [j, t]; head-outer
                        # so head-0's rope completes before any head-1 work
                        # and head-0 attention overlaps the head-1 tail.
                        for m in range(HPC):
                            for wc, acc, rope_dst in (
                                (wq_c, qacc, qrope),
                                (wk_c, kacc, krope),
                            ):
                                for s in range(NS):
                                    ps = psQK.tile(
                                        [128, 512], F32, tag="qk_ps", name="qk_ps"
                                    )
                                    for i in range(CH):
                                        nc.tensor.matmul(
                                            ps[:],
                                            wc[:, ds(i * JC + m * D, D)],
                                            xh[i][:, ts(s, 512)],
                                            start=(i == 0),
                                            stop=(i == CH - 1),
                                        )
                                    asl = acc[:, ds(m * S + s * 512, 512)]
                                    if first:
                                        nc.vector.tensor_copy(asl, ps[:])
                                    else:
                                        nc.vector.scalar_tensor_tensor(
                                            asl, ps[:], 1.0, asl, ALU.mult, ALU.add
                                        )
                                        # RoPE: r = cos*q + sin*swap64(q)
                                        sw = rsp.tile(
                                            [128, 512], F32, tag="rsw", name="rsw"
                                        )
                                        nc.vector.tensor_copy(
                                            sw[0:64, :], asl[64:128, :]
                                        )
                                        nc.vector.tensor_copy(
                                            sw[64:128, :], asl[0:64, :]
                                        )
                                        nc.vector.tensor_mul(
                                            asl, asl, cos_sb[:, ts(s, 512)]
                                        )
                                        nc.vector.tensor_mul(
                                            sw[:], sw[:], sin_sb[:, ts(s, 512)]
                                        )
                                        nc.vector.tensor_tensor(
                                            rope_dst[:, ds(m * S + s * 512, 512)],
                                            asl,
                                            sw[:],
                                            ALU.add,
                                        )

                    # pass 1: e-tiles 8..15 (resident only once)
                    w1 = load_weights(1)
                    xh1 = stream_xs(1, first=True)
                    qkv_chunk(xh1, *w1, first=True)

                    # pass 2: e-tiles 0..7 + rms + rope + v scale
                    w0 = load_weights(0)
                    xh0 = stream_xs(0, first=False)
                    for s in range(NS):
                        # rms = sqrt(ssq/E + eps)
                        nc.scalar.activation(
                            rms_row[0:1, ts(s, 512)],
                            ssq_ps[s][:],
                            AF.Sqrt,
                            bias=eps_sc[0:1, 0:1],
                            scale=1.0 / E,
                        )
                    nc.vector.reciprocal(rms_row[:], rms_row[:])
                    nc.gpsimd.partition_broadcast(bcastR[:], rms_row[0:1, :])
                    # token-major view of the recips for the v scaling;
                    # bounce via DRAM so the strided gather runs on the
                    # DRAM side of the DMA.
                    rrow_d = dpool.tile([1, S], F32, tag="rrow_d", name="rrow_d")
                    nc.sync.dma_start(rrow_d[:], rms_row[:])
                    nc.sync.dma_start(
                        recip_col[:],
                        rrow_d[0, :].rearrange("(a p) -> p a", p=128),
                    )
                    # fold 1/rms into the rope tables (before pass-2 rope)
                    nc.vector.tensor_mul(cos_sb[:], cos_sb[:], bcastR[:])
                    nc.vector.tensor_mul(sin_sb[:], sin_sb[:], bcastR[:])
                    qkv_chunk(xh0, *w0, first=False)

            # ---------------- Phase D: attention ----------------
            with (
                tc.tile_pool(name="attn", bufs=1) as apl,
                tc.tile_pool(name="probs", bufs=8) as prp,
                tc.tile_pool(name="bcD", bufs=2) as bdp,
                tc.tile_pool(name="psS", bufs=4, space="PSUM") as psS,
                tc.tile_pool(name="psCtx", bufs=2, space="PSUM") as psC,
                tc.tile_pool(name="psSum", bufs=2, space="PSUM") as psU,
            ):
                ctx_sb = apl.tile([128, HPC * S], BF16, tag="ctx_sb")
                # single lower-triangle mask tile: tri[i, c] = 1 iff i <= c
                tri = apl.tile([128, 128], F32, tag="tri")
                nc.sync.dma_start(
                    tri[:],
                    blob[ds(OFF_TRI, NE_TRI)]
                    .bitcast(F32)
                    .rearrange("(p c) -> p c", p=128),
                )

                for m in range(HPC):
                    for s in range(NS):
                        n_tk = 4 * (s + 1)
                        ctx_ps = psC.tile([128, 512], F32, tag="ctx_ps", name="ctx_ps")
                        sum_ps = psU.tile([1, 512], F32, tag="sum_ps", name="sum_ps")
                        for j in range(n_tk):
                            p_rel = j - 4 * s
                            # diagonal blocks only attend to tq_local >= off
                            off = 128 * p_rel if p_rel >= 0 else 0
                            n = 512 - off
                            sc = psS.tile([128, 512], F32, tag="sc", name="sc")
                            nc.tensor.matmul(
                                sc[:, 0:n],
                                krope[:, ds(m * S + j * 128, 128)],
                                qrope[:, ds(m * S + s * 512 + off, n)],
                                start=True,
                                stop=True,
                            )
                            pr = prp.tile([128, 512], F32R, tag="probs", name="pr")
                            if p_rel >= 0:
                                # triangle (first 128 cols of the valid range)
                                et = prp.tile([128, 128], F32, tag="expt", name="et")
                                nc.scalar.activation(
                                    et[:], sc[:, 0:128], AF.Exp, scale=INV_SQRT_D
                                )
                                nc.vector.tensor_mul(pr[:, 0:128], et[:], tri[:])
                                if n > 128:
                                    nc.scalar.activation(
                                        pr[:, 128:n],
                                        sc[:, 128:n],
                                        AF.Exp,
                                        scale=INV_SQRT_D,
                                    )
                            else:
                                nc.scalar.activation(
                                    pr[:, 0:n], sc[:, 0:n], AF.Exp, scale=INV_SQRT_D
                                )
                            nc.tensor.matmul(
                                ctx_ps[:, ds(off, n)],
                                v_sb[:, ds(j * JC + m * D, D)],
                                pr[:, 0:n],
                                start=(j == 0),
                                stop=(j == n_tk - 1),
                            )
                            nc.tensor.matmul(
                                sum_ps[0:1, ds(off, n)],
                                ones_r[:],
                                pr[:, 0:n],
                                start=(j == 0),
                                stop=(j == n_tk - 1),
                            )
                        rr = bdp.tile([1, 512], F32, tag="recip", name="rr")
                        nc.vector.reciprocal(rr[:], sum_ps[:])
                        bc = bdp.tile([128, 512], F32, tag="bcD", name="bc")
                        nc.gpsimd.partition_broadcast(bc[:], rr[0:1, :])
                        nc.vector.tensor_mul(
                            ctx_sb[:, ds(m * S + s * 512, 512)], ctx_ps[:], bc[:]
                        )
                        nc.sync.dma_start(
                            cbounce[m][:, ts(s, 512)],
                            ctx_sb[:, ds(m * S + s * 512, 512)],
                        )
                    nc.gpsimd.collective_compute(
                        "AllGather",
                        ALU.bypass,
                        replica_groups=rg,
                        ins=[cbounce[m].opt()],
                        outs=[ag_out[m].opt()],
                    )

            # ---------------- Phase E: output projection ----------------
            # Two passes: the even-head half (ag_out[0]) contracts as soon as
            # the first AllGather lands — overlapping head-1 attention and
            # the second AllGather — with partials parked in SBUF; the odd
            # half then adds on top and evicts.
            with (
                tc.tile_pool(name="ck", bufs=ET) as ckp,
                tc.tile_pool(name="wo", bufs=1) as wop,
                tc.tile_pool(name="ob", bufs=2) as obp,
                tc.tile_pool(name="ob1", bufs=1) as ob1p,
                tc.tile_pool(name="psW", bufs=3, space="PSUM") as psW,
            ):
                woT_r = wop.tile([128, ET * EB], F32R, tag="woT_r")
                nc.sync.dma_start(
                    woT_r[:].rearrange("p (a j) -> p a j", a=ET),
                    blob[ds(OFF_WO, NE_W)].rearrange(
                        "(a p j) -> p a j", p=128, j=EB
                    ),
                )
                woT_sb = wop.tile([128, ET * EB], BF16, tag="woT_sb")
                nc.vector.tensor_copy(woT_sb[:], woT_r[:].bitcast(F32))
                ctxk = []
                for kb in range(ET):
                    ct = ckp.tile([128, S], BF16, tag="ck", name=f"ck{kb}")
                    src = ag_out[0] if kb < CH else ag_out[1]
                    nc.sync.dma_start(ct[:], src[ts(kb % CH, 128), :])
                    ctxk.append(ct)
                ob1 = ob1p.tile([128, TT * EB], F32, tag="ob1")
                for t in range(TT):
                    ps = psW.tile([128, EB], F32, tag="wo_ps", name="wo_ps")
                    for kb in range(CH):
                        nc.tensor.matmul(
                            ps[:],
                            ctxk[kb][:, ts(t, 128)],
                            woT_sb[:, ts(kb, EB)],
                            start=(kb == 0),
                            stop=(kb == CH - 1),
                        )
                    nc.vector.tensor_copy(ob1[:, ts(t, EB)], ps[:])
                for t in range(TT):
                    ps = psW.tile([128, EB], F32, tag="wo_ps", name="wo_ps")
                    for kb in range(CH, ET):
                        nc.tensor.matmul(
                            ps[:],
                            ctxk[kb][:, ts(t, 128)],
                            woT_sb[:, ts(kb, EB)],
                            start=(kb == CH),
                            stop=(kb == ET - 1),
                        )
                    ob = obp.tile([128, EB], F32, tag="ob", name="ob")
                    nc.vector.scalar_tensor_tensor(
                        ob[:], ps[:], 1.0, ob1[:, ts(t, EB)], ALU.mult, ALU.add
                    )
                    nc.sync.dma_start(out_ext[ts(t, 128), :], ob[:])

    nc.compile()
    return nc


def get_nc():
    if "nc" not in _NC_CACHE:
        _NC_CACHE["nc"] = _build_nc()
    return _NC_CACHE["nc"]


def _round_f32r(a):
    """Round fp32 to fp32r (11 explicit mantissa bits) with RNE."""
    u = np.ascontiguousarray(a, dtype=np.float32).view(np.uint32).copy()
    round_bit = (u >> 12) & 1
    u += 0x7FF + round_bit
    u &= np.uint32(0xFFFFF000)
    return u.view(np.float32)


def _rope_tables():
    """thetas with the reference's fp16-arange quirk, then f32 cos/sin."""
    try:
        # Same ops/dtypes as the reference, on the default jax device, so
        # the fp16 pow rounds identically to the reference run in this env.
        import jax.numpy as jnp

        th = (
            THETA ** (-jnp.arange(HALF, dtype=jnp.float16) / HALF)
        ).astype(jnp.float32)
        thetas = np.asarray(th)
    except Exception:
        ar = np.arange(HALF, dtype=np.float16)
        y = -ar / np.float16(HALF)
        thetas = (np.float16(THETA) ** y).astype(np.float32)
    m = np.arange(S, dtype=np.float32)
    ang = m[:, None] * thetas[None, :]  # [S, 64] f32
    cos = np.ascontiguousarray(np.cos(ang).astype(np.float32).T)  # [64, S]
    sin = np.ascontiguousarray(np.sin(ang).astype(np.float32).T)
    cosF = np.concatenate([cos, cos], axis=0)  # [128, S]
    sinF = np.concatenate([-sin, sin], axis=0)
    return np.ascontiguousarray(cosF), np.ascontiguousarray(sinF)


def _host_prep(xs, norm_w, wq, wk, wv, wo):
    xs = np.asarray(xs, dtype=np.float32)
    norm_w = np.asarray(norm_w, dtype=np.float32)
    wq = np.asarray(wq, dtype=np.float32)
    wk = np.asarray(wk, dtype=np.float32)
    wv = np.asarray(wv, dtype=np.float32)
    wo = np.asarray(wo, dtype=np.float32)

    xsT = _round_f32r(np.ascontiguousarray(xs.T))
    cosF, sinF = _rope_tables()

    tri = (
        np.arange(128, dtype=np.int64)[:, None]
        <= np.arange(128, dtype=np.int64)[None, :]
    ).astype(np.float32)

    perm = np.concatenate([np.arange(0, D, 2), np.arange(1, D, 2)])
    wq_n = wq * norm_w[None, :]
    wk_n = wk * norm_w[None, :]
    wv_n = wv * norm_w[None, :]
    f_order = np.concatenate(
        [np.arange(h * D, (h + 1) * D) for h in range(0, H, 2)]
        + [np.arange(h * D, (h + 1) * D) for h in range(1, H, 2)]
    )

    in_maps = []
    for c in range(N_CORES):
        heads = (2 * c, 2 * c + 1)
        rows_qk = np.concatenate([h * D + perm for h in heads])
        rows_v = np.concatenate([np.arange(h * D, (h + 1) * D) for h in heads])
        blob = np.empty(NE_TOT, dtype=np.float32)
        blob[OFF_XS:OFF_WQ] = xsT.ravel()
        blob[OFF_WQ:OFF_WK] = _round_f32r(
            np.ascontiguousarray(wq_n[rows_qk].T)
        ).ravel()
        blob[OFF_WK:OFF_WV] = _round_f32r(
            np.ascontiguousarray(wk_n[rows_qk].T)
        ).ravel()
        blob[OFF_WV:OFF_WO] = _round_f32r(
            np.ascontiguousarray(wv_n[rows_v].T)
        ).ravel()
        blob[OFF_WO:OFF_COS] = _round_f32r(
            np.ascontiguousarray(wo[c * EB : (c + 1) * EB, :].T[f_order, :])
        ).ravel()
        blob[OFF_COS:OFF_SIN] = cosF.ravel()
        blob[OFF_SIN:OFF_TRI] = sinF.ravel()
        blob[OFF_TRI:NE_TOT] = tri.ravel()
        in_maps.append({"blob": blob})
    return in_maps


def kernel(xs, norm_w, wq, wk, wv, wo):
    from concourse.bass_utils import run_bass_kernel_spmd

    nc = get_nc()
    in_maps = _host_prep(xs, norm_w, wq, wk, wv, wo)
    res = run_bass_kernel_spmd(nc, in_maps, list(range(N_CORES)))
    out = np.concatenate([res.results[c]["out"] for c in range(N_CORES)], axis=1)
    return out.astype(np.float32)



# revision 27
# speedup vs baseline: 1.1932x; 1.1932x over previous
"""Trainium2 Bass kernel for nn_AttentionModule_53223234732422.

Computes: RMSNorm -> QKV projections -> interleaved-pair RoPE on Q,K ->
causal softmax attention (16 heads, head_dim 128) -> output projection.

Sharding (8 NeuronCores, tensor parallel over heads):
  - every core computes the RMSNorm (cheap, avoids an activation collective),
  - each core owns 2 heads: QKV projections with column-sliced weights,
    RoPE, causal attention for those heads,
  - per-head context is AllGathered in bf16 (2 x 0.5 MiB per rank); the
    output projection runs bf16 x bf16 (wo converted on device),
  - output projection is split column-wise: each core produces 256 output
    features from the full gathered context.

Host-side preparation (layout only):
  - ALL inputs are packed into a single flat f32r dram blob per core
    (xsT | wqT | wkT | wvT | woT | cos | sin | tri): each extra PJRT
    operand costs measurable per-execute marshaling time under axon,
  - xs transposed to feature-major [E, S] so contractions land on SBUF
    partitions,
  - norm_w folded into the QKV weights,
  - wq/wk rows permuted per head so RoPE pairs are deinterleaved
    (x0 rows 0..63, x1 rows 64..127); scores are permutation invariant,
  - weights pre-rounded to fp32r (11 mantissa bits, RNE) to match the
    on-device rounding path,
  - cos/sin tables (fp16-arange thetas, like the reference) and the 4
    diagonal causal-mask tiles precomputed.

Dtypes: all matmuls run fp32r (full-rate fp32 path on the PE, 11 mantissa
bits, ~1.2e-4 input rounding; plain fp32 would be 4x slower). PSUM
accumulation is fp32 everywhere. Every tensor an fp32r matmul consumes is
written only by fp32r-typed producers (BIR verifier requirement); weights
are pre-rounded on the host and DMA'd with fp32r-typed endpoints.
Measured on HW: end-to-end relative error 2.4e-3 vs the fp32 reference
(fp32r matmul rounding ~2e-4 + bf16 context/wo ~2e-3; gate is 2e-2).
"""

import sys

sys.path.insert(0, "/opt/trn_rl_repo")

import numpy as np

import concourse.bacc as bacc
import concourse.mybir as mybir
import concourse.tile as tile
from concourse.bass import ds, ts

dt = mybir.dt
AF = mybir.ActivationFunctionType
ALU = mybir.AluOpType

S = 2048
E = 2048
H = 16
D = 128
HALF = D // 2
EPS = 1e-6
THETA = 10000.0
N_CORES = 8
HPC = H // N_CORES  # heads per core
JC = HPC * D  # 256: local q/k/v width
EB = E // N_CORES  # 256: output columns per core
ET = E // 128  # 16 feature tiles
TT = S // 128  # 16 token tiles
NS = S // 512  # 4 token strips
CH = ET // 2  # 8 e-tiles per contraction chunk
INV_SQRT_D = float(1.0 / np.sqrt(np.float32(D)))

F32 = dt.float32
F32R = dt.float32r
BF16 = dt.bfloat16

# Single packed input blob (element offsets, f32/f32r are both 4 bytes).
# One ExternalInput instead of eight: each extra operand costs real
# per-execute marshaling time in the axon PJRT dispatch path.
NE_XS = E * S
NE_W = E * JC  # == E * EB for woT
NE_TRIG = D * S
NE_TRI = 128 * 128
OFF_XS = 0
OFF_WQ = OFF_XS + NE_XS
OFF_WK = OFF_WQ + NE_W
OFF_WV = OFF_WK + NE_W
OFF_WO = OFF_WV + NE_W
OFF_COS = OFF_WO + NE_W
OFF_SIN = OFF_COS + NE_TRIG
OFF_TRI = OFF_SIN + NE_TRIG
NE_TOT = OFF_TRI + NE_TRI

_NC_CACHE = {}


def _build_nc():
    nc = bacc.Bacc(trn_type="TRN2", num_devices=N_CORES)

    blob = nc.dram_tensor("blob", [NE_TOT], F32R, kind="ExternalInput")
    out_ext = nc.dram_tensor("out", [S, EB], F32, kind="ExternalOutput")

    def xs_rows(e):
        return blob[ds(OFF_XS + e * 128 * S, 128 * S)].rearrange(
            "(p t) -> p t", p=128
        )

    def w_chunk(base, chunk):
        return blob[ds(base + chunk * CH * 128 * JC, CH * 128 * JC)].rearrange(
            "(a p j) -> p a j", p=128, j=JC
        )

    def trig(base):
        return (
            blob[ds(base, NE_TRIG)]
            .bitcast(F32)
            .rearrange("(p t) -> p t", p=128)
        )

    rg = [list(range(N_CORES))]

    with tile.TileContext(nc) as tc:
        with (
            tc.tile_pool(name="persist", bufs=1) as pp,
            tc.tile_pool(name="dram", bufs=1, space="DRAM") as dpool,
        ):
            ones_f = pp.tile([128, 1], F32, tag="ones_f")
            ones_r = pp.tile([128, 1], F32R, tag="ones_r")
            ones_b = pp.tile([128, 1], BF16, tag="ones_b")
            eps_sc = pp.tile([1, 1], F32, tag="eps_sc")
            nc.vector.memset(ones_f[:], 1.0)
            nc.vector.tensor_copy(ones_r[:], ones_f[:])
            nc.vector.tensor_copy(ones_b[:], ones_f[:])
            nc.vector.memset(eps_sc[:], EPS)

            # RoPE'd q/k (fp32r; written only by the final rope add) and
            # bf16 token-major v.
            qrope = pp.tile([128, HPC * S], F32R, tag="qrope")
            krope = pp.tile([128, HPC * S], F32R, tag="krope")
            v_sb = pp.tile([128, TT * JC], F32R, tag="v_sb")

            # context is gathered in bf16: halves the AllGather payload and
            # the Phase-E HBM reads; output error stays ~2-4e-3 (gate 2e-2)
            cbounce = [
                dpool.tile([128, S], BF16, tag=f"cb{m}", name=f"cb{m}")
                for m in range(HPC)
            ]
            ag_out = [
                dpool.tile(
                    [N_CORES * 128, S],
                    BF16,
                    addr_space="Shared",
                    tag=f"ag{m}",
                    name=f"ag{m}",
                )
                for m in range(HPC)
            ]

            # ---------- Phase A+C: fused RMS + QKV, xs read ONCE -----------
            # The 1/rms per-token scale commutes out of the e-contraction, so
            # QKV consumes RAW xs (host-pre-rounded to fp32r); the scale is
            # folded into the cos/sin tables (q/k) and a v post-pass.
            # Pass 1 streams e-tiles 8..15, squares them for the running ssq
            # AND contracts them for QKV while resident; pass 2 does the same
            # for tiles 0..7, then computes rms, folds it into the rope
            # tables, ropes q/k, and scales v. This avoids the 8 MiB chunk-1
            # re-read the 2-phase version paid.
            with tc.tile_pool(name="bcC", bufs=1) as bcp:
                bcastR = bcp.tile([128, S], F32, tag="bcastR")
                recip_col = bcp.tile([128, TT], F32, tag="recip_col")
                with (
                    tc.tile_pool(name="xsp", bufs=CH) as xsp,
                    tc.tile_pool(name="rmsp", bufs=1) as rmsp,
                    tc.tile_pool(name="sqp", bufs=2) as sqp,
                    tc.tile_pool(name="wch", bufs=3) as wchp,
                    tc.tile_pool(name="acc", bufs=1) as accp,
                    tc.tile_pool(name="trig", bufs=1) as trigp,
                    tc.tile_pool(name="rsw", bufs=1) as rsp,
                    tc.tile_pool(name="psA", bufs=NS, space="PSUM") as psA,
                    tc.tile_pool(name="psQK", bufs=2, space="PSUM") as psQK,
                    tc.tile_pool(name="psV", bufs=2, space="PSUM") as psV,
                ):
                    rms_row = rmsp.tile([1, S], F32, tag="rms_row")
                    ssq_ps = [
                        psA.tile([1, 512], F32, tag="ssq", name="ssq")
                        for _ in range(NS)
                    ]
                    cos_sb = trigp.tile([D, S], F32, tag="cos_sb")
                    sin_sb = trigp.tile([D, S], F32, tag="sin_sb")
                    nc.sync.dma_start(cos_sb[:], trig(OFF_COS))
                    nc.sync.dma_start(sin_sb[:], trig(OFF_SIN))
                    qacc = accp.tile([128, HPC * S], F32, tag="qacc")
                    kacc = accp.tile([128, HPC * S], F32, tag="kacc")

                    def load_weights(chunk):
                        wtiles = []
                        for wname, wbase in (
                            ("wq", OFF_WQ),
                            ("wk", OFF_WK),
                            ("wv", OFF_WV),
                        ):
                            wc = wchp.tile(
                                [128, CH * JC], F32R, tag="wch",
                                name=f"w{chunk}_{wname}",
                            )
                            nc.sync.dma_start(
                                wc[:].rearrange("p (a j) -> p a j", a=CH),
                                w_chunk(wbase, chunk),
                            )
                            wtiles.append(wc)
                        return wtiles

                    def stream_xs(chunk, first):
                        xh = []
                        for i in range(CH):
                            e = chunk * CH + i
                            xt = xsp.tile([128, S], F32R, tag="xsA", name="xsA")
                            nc.sync.dma_start(xt[:], xs_rows(e))
                            xh.append(xt)
                            for s in range(NS):
                                sq = sqp.tile(
                                    [128, 512], F32R, tag="sq", name="sq"
                                )
                                nc.vector.tensor_mul(
                                    sq[:],
                                    xt[:, ts(s, 512)].bitcast(F32),
                                    xt[:, ts(s, 512)].bitcast(F32),
                                )
                                nc.tensor.matmul(
                                    ssq_ps[s][:],
                                    ones_r[:],
                                    sq[:],
                                    start=(first and i == 0),
                                    stop=((not first) and i == CH - 1),
                                )
                        return xh

                    def qkv_chunk(xh, wq_c, wk_c, wv_c, first):
                        # v projection FIRST -> token-major [t, j], raw sum
                        # (the 1/rms scale is applied in a post-pass):
                        # attention consumes v, so finishing it early lets
                        # head-0 attention start under the q/k tail.
                        for t in range(TT):
                            ps = psV.tile([128, JC], F32, tag="v_ps", name="v_ps")
                            for i in range(CH):
                                nc.tensor.matmul(
                                    ps[:],
                                    xh[i][:, ts(t, 128)],
                                    wv_c[:, ts(i, JC)],
                                    start=(i == 0),
                                    stop=(i == CH - 1),
                                )
                            vsl = v_sb[:, ts(t, JC)]
                            if first:
                                nc.vector.tensor_copy(vsl, ps[:])
                            else:
                                nc.vector.scalar_tensor_tensor(
                                    vsl,
                                    ps[:],
                                    1.0,
                                    vsl.bitcast(F32),
                                    ALU.mult,
                                    ALU.add,
                                )
                        if not first:
                            # apply the per-token 1/rms right away so v is
                            # attention-ready before the rope DVE work queues
                            for t in range(TT):
                                vsl = v_sb[:, ts(t, JC)]
                                nc.vector.tensor_scalar_mul(
                                    vsl, vsl.bitcast(F32), recip_col[:, t : t + 1]
                                )

                        # q and k projections -> d-major [j, t]; head-outer
                        # so head-0's rope completes before any head-1 work
                        # and head-0 attention overlaps the head-1 tail.
                        for m in range(HPC):
                            for wc, acc, rope_dst in (
                                (wq_c, qacc, qrope),
                                (wk_c, kacc, krope),
                            ):
                                for s in range(NS):
                                    ps = psQK.tile(
                                        [128, 512], F32, tag="qk_ps", name="qk_ps"
                                    )
                                    for i in range(CH):
                                        nc.tensor.matmul(
                                            ps[:],
                                            wc[:, ds(i * JC + m * D, D)],
                                            xh[i][:, ts(s, 512)],
                                            start=(i == 0),
                                            stop=(i == CH - 1),
                                        )
                                    asl = acc[:, ds(m * S + s * 512, 512)]
                                    if first:
                                        nc.vector.tensor_copy(asl, ps[:])
                                    else:
                                        nc.vector.scalar_tensor_tensor(
                                            asl, ps[:], 1.0, asl, ALU.mult, ALU.add
                                        )
                                        # RoPE: r = cos*q + sin*swap64(q)
                                        sw = rsp.tile(
                                            [128, 512], F32, tag="rsw", name="rsw"
                                        )
                                        nc.vector.tensor_copy(
                                            sw[0:64, :], asl[64:128, :]
                                        )
                                        nc.vector.tensor_copy(
                                            sw[64:128, :], asl[0:64, :]
                                        )
                                        nc.vector.tensor_mul(
                                            asl, asl, cos_sb[:, ts(s, 512)]
                                        )
                                        nc.vector.tensor_mul(
                                            sw[:], sw[:], sin_sb[:, ts(s, 512)]
                                        )
                                        nc.vector.tensor_tensor(
                                            rope_dst[:, ds(m * S + s * 512, 512)],
                                            asl,
                                            sw[:],
                                            ALU.add,
                                        )

                    # pass 1: e-tiles 8..15 (resident only once)
                    w1 = load_weights(1)
                    xh1 = stream_xs(1, first=True)
                    qkv_chunk(xh1, *w1, first=True)

                    # pass 2: e-tiles 0..7 + rms + rope + v scale
                    w0 = load_weights(0)
                    xh0 = stream_xs(0, first=False)
                    for s in range(NS):
                        # rms = sqrt(ssq/E + eps)
                        nc.scalar.activation(
                            rms_row[0:1, ts(s, 512)],
                            ssq_ps[s][:],
                            AF.Sqrt,
                            bias=eps_sc[0:1, 0:1],
                            scale=1.0 / E,
                        )
                    nc.vector.reciprocal(rms_row[:], rms_row[:])
                    nc.gpsimd.partition_broadcast(bcastR[:], rms_row[0:1, :])
                    # token-major view of the recips for the v scaling;
                    # bounce via DRAM so the strided gather runs on the
                    # DRAM side of the DMA.
                    rrow_d = dpool.tile([1, S], F32, tag="rrow_d", name="rrow_d")
                    nc.sync.dma_start(rrow_d[:], rms_row[:])
                    nc.sync.dma_start(
                        recip_col[:],
                        rrow_d[0, :].rearrange("(a p) -> p a", p=128),
                    )
                    # fold 1/rms into the rope tables (before pass-2 rope)
                    nc.vector.tensor_mul(cos_sb[:], cos_sb[:], bcastR[:])
                    nc.vector.tensor_mul(sin_sb[:], sin_sb[:], bcastR[:])
                    qkv_chunk(xh0, *w0, first=False)

            # ---------------- Phase D: attention ----------------
            with (
                tc.tile_pool(name="attn", bufs=1) as apl,
                tc.tile_pool(name="probs", bufs=8) as prp,
                tc.tile_pool(name="bcD", bufs=2) as bdp,
                tc.tile_pool(name="psS", bufs=4, space="PSUM") as psS,
                tc.tile_pool(name="psCtx", bufs=2, space="PSUM") as psC,
                tc.tile_pool(name="psSum", bufs=2, space="PSUM") as psU,
            ):
                ctx_sb = apl.tile([128, HPC * S], BF16, tag="ctx_sb")
                # single lower-triangle mask tile: tri[i, c] = 1 iff i <= c
                tri = apl.tile([128, 128], F32, tag="tri")
                nc.sync.dma_start(
                    tri[:],
                    blob[ds(OFF_TRI, NE_TRI)]
                    .bitcast(F32)
                    .rearrange("(p c) -> p c", p=128),
                )

                for m in range(HPC):
                    for s in range(NS):
                        n_tk = 4 * (s + 1)
                        ctx_ps = psC.tile([128, 512], F32, tag="ctx_ps", name="ctx_ps")
                        sum_ps = psU.tile([1, 512], F32, tag="sum_ps", name="sum_ps")
                        for j in range(n_tk):
                            p_rel = j - 4 * s
                            # diagonal blocks only attend to tq_local >= off
                            off = 128 * p_rel if p_rel >= 0 else 0
                            n = 512 - off
                            sc = psS.tile([128, 512], F32, tag="sc", name="sc")
                            nc.tensor.matmul(
                                sc[:, 0:n],
                                krope[:, ds(m * S + j * 128, 128)],
                                qrope[:, ds(m * S + s * 512 + off, n)],
                                start=True,
                                stop=True,
                            )
                            pr = prp.tile([128, 512], F32R, tag="probs", name="pr")
                            if p_rel >= 0:
                                # triangle (first 128 cols of the valid range)
                                et = prp.tile([128, 128], F32, tag="expt", name="et")
                                nc.scalar.activation(
                                    et[:], sc[:, 0:128], AF.Exp, scale=INV_SQRT_D
                                )
                                nc.vector.tensor_mul(pr[:, 0:128], et[:], tri[:])
                                if n > 128:
                                    nc.scalar.activation(
                                        pr[:, 128:n],
                                        sc[:, 128:n],
                                        AF.Exp,
                                        scale=INV_SQRT_D,
                                    )
                            else:
                                nc.scalar.activation(
                                    pr[:, 0:n], sc[:, 0:n], AF.Exp, scale=INV_SQRT_D
                                )
                            nc.tensor.matmul(
                                ctx_ps[:, ds(off, n)],
                                v_sb[:, ds(j * JC + m * D, D)],
                                pr[:, 0:n],
                                start=(j == 0),
                                stop=(j == n_tk - 1),
                            )
                            nc.tensor.matmul(
                                sum_ps[0:1, ds(off, n)],
                                ones_r[:],
                                pr[:, 0:n],
                                start=(j == 0),
                                stop=(j == n_tk - 1),
                            )
                        rr = bdp.tile([1, 512], F32, tag="recip", name="rr")
                        nc.vector.reciprocal(rr[:], sum_ps[:])
                        bc = bdp.tile([128, 512], F32, tag="bcD", name="bc")
                        nc.gpsimd.partition_broadcast(bc[:], rr[0:1, :])
                        nc.vector.tensor_mul(
                            ctx_sb[:, ds(m * S + s * 512, 512)], ctx_ps[:], bc[:]
                        )
                        nc.sync.dma_start(
                            cbounce[m][:, ts(s, 512)],
                            ctx_sb[:, ds(m * S + s * 512, 512)],
                        )
                    nc.gpsimd.collective_compute(
                        "AllGather",
                        ALU.bypass,
                        replica_groups=rg,
                        ins=[cbounce[m].opt()],
                        outs=[ag_out[m].opt()],
                    )

            # ---------------- Phase E: output projection ----------------
            # Two passes: the even-head half (ag_out[0]) contracts as soon as
            # the first AllGather lands — overlapping head-1 attention and
            # the second AllGather — with partials parked in SBUF; the odd
            # half then adds on top and evicts.
            with (
                tc.tile_pool(name="ck", bufs=ET) as ckp,
                tc.tile_pool(name="wo", bufs=1) as wop,
                tc.tile_pool(name="ob", bufs=2) as obp,
                tc.tile_pool(name="ob1", bufs=1) as ob1p,
                tc.tile_pool(name="psW", bufs=3, space="PSUM") as psW,
            ):
                woT_r = wop.tile([128, ET * EB], F32R, tag="woT_r")
                nc.sync.dma_start(
                    woT_r[:].rearrange("p (a j) -> p a j", a=ET),
                    blob[ds(OFF_WO, NE_W)].rearrange(
                        "(a p j) -> p a j", p=128, j=EB
                    ),
                )
                woT_sb = wop.tile([128, ET * EB], BF16, tag="woT_sb")
                nc.vector.tensor_copy(woT_sb[:], woT_r[:].bitcast(F32))
                ctxk = []
                for kb in range(ET):
                    ct = ckp.tile([128, S], BF16, tag="ck", name=f"ck{kb}")
                    src = ag_out[0] if kb < CH else ag_out[1]
                    nc.sync.dma_start(ct[:], src[ts(kb % CH, 128), :])
                    ctxk.append(ct)
                ob1 = ob1p.tile([128, TT * EB], F32, tag="ob1")
                for t in range(TT):
                    ps = psW.tile([128, EB], F32, tag="wo_ps", name="wo_ps")
                    for kb in range(CH):
                        nc.tensor.matmul(
                            ps[:],
                            ctxk[kb][:, ts(t, 128)],
                            woT_sb[:, ts(kb, EB)],
                            start=(kb == 0),
                            stop=(kb == CH - 1),
                        )
                    nc.vector.tensor_copy(ob1[:, ts(t, EB)], ps[:])
                for t in range(TT):
                    ps = psW.tile([128, EB], F32, tag="wo_ps", name="wo_ps")
                    for kb in range(CH, ET):
                        nc.tensor.matmul(
                            ps[:],
                            ctxk[kb][:, ts(t, 128)],
                            woT_sb[:, ts(kb, EB)],
                            start=(kb == CH),
                            stop=(kb == ET - 1),
                        )
                    ob = obp.tile([128, EB], F32, tag="ob", name="ob")
                    nc.vector.scalar_tensor_tensor(
                        ob[:], ps[:], 1.0, ob1[:, ts(t, EB)], ALU.mult, ALU.add
                    )
                    nc.sync.dma_start(out_ext[ts(t, 128), :], ob[:])

    nc.compile()
    return nc


def get_nc():
    if "nc" not in _NC_CACHE:
        _NC_CACHE["nc"] = _build_nc()
    return _NC_CACHE["nc"]


def _round_f32r(a):
    """Round fp32 to fp32r (11 explicit mantissa bits) with RNE."""
    u = np.ascontiguousarray(a, dtype=np.float32).view(np.uint32).copy()
    round_bit = (u >> 12) & 1
    u += 0x7FF + round_bit
    u &= np.uint32(0xFFFFF000)
    return u.view(np.float32)


def _rope_tables():
    """thetas with the reference's fp16-arange quirk, then f32 cos/sin."""
    try:
        # Same ops/dtypes as the reference, on the default jax device, so
        # the fp16 pow rounds identically to the reference run in this env.
        import jax.numpy as jnp

        th = (
            THETA ** (-jnp.arange(HALF, dtype=jnp.float16) / HALF)
        ).astype(jnp.float32)
        thetas = np.asarray(th)
    except Exception:
        ar = np.arange(HALF, dtype=np.float16)
        y = -ar / np.float16(HALF)
        thetas = (np.float16(THETA) ** y).astype(np.float32)
    m = np.arange(S, dtype=np.float32)
    ang = m[:, None] * thetas[None, :]  # [S, 64] f32
    cos = np.ascontiguousarray(np.cos(ang).astype(np.float32).T)  # [64, S]
    sin = np.ascontiguousarray(np.sin(ang).astype(np.float32).T)
    cosF = np.concatenate([cos, cos], axis=0)  # [128, S]
    sinF = np.concatenate([-sin, sin], axis=0)
    return np.ascontiguousarray(cosF), np.ascontiguousarray(sinF)


def _host_prep(xs, norm_w, wq, wk, wv, wo):
    xs = np.asarray(xs, dtype=np.float32)
    norm_w = np.asarray(norm_w, dtype=np.float32)
    wq = np.asarray(wq, dtype=np.float32)
    wk = np.asarray(wk, dtype=np.float32)
    wv = np.asarray(wv, dtype=np.float32)
    wo = np.asarray(wo, dtype=np.float32)

    xsT = _round_f32r(np.ascontiguousarray(xs.T))
    cosF, sinF = _rope_tables()

    tri = (
        np.arange(128, dtype=np.int64)[:, None]
        <= np.arange(128, dtype=np.int64)[None, :]
    ).astype(np.float32)

    perm = np.concatenate([np.arange(0, D, 2), np.arange(1, D, 2)])
    wq_n = wq * norm_w[None, :]
    wk_n = wk * norm_w[None, :]
    wv_n = wv * norm_w[None, :]
    f_order = np.concatenate(
        [np.arange(h * D, (h + 1) * D) for h in range(0, H, 2)]
        + [np.arange(h * D, (h + 1) * D) for h in range(1, H, 2)]
    )

    in_maps = []
    for c in range(N_CORES):
        heads = (2 * c, 2 * c + 1)
        rows_qk = np.concatenate([h * D + perm for h in heads])
        rows_v = np.concatenate([np.arange(h * D, (h + 1) * D) for h in heads])
        blob = np.empty(NE_TOT, dtype=np.float32)
        blob[OFF_XS:OFF_WQ] = xsT.ravel()
        blob[OFF_WQ:OFF_WK] = _round_f32r(
            np.ascontiguousarray(wq_n[rows_qk].T)
        ).ravel()
        blob[OFF_WK:OFF_WV] = _round_f32r(
            np.ascontiguousarray(wk_n[rows_qk].T)
        ).ravel()
        blob[OFF_WV:OFF_WO] = _round_f32r(
            np.ascontiguousarray(wv_n[rows_v].T)
        ).ravel()
        blob[OFF_WO:OFF_COS] = _round_f32r(
            np.ascontiguousarray(wo[c * EB : (c + 1) * EB, :].T[f_order, :])
        ).ravel()
        blob[OFF_COS:OFF_SIN] = cosF.ravel()
        blob[OFF_SIN:OFF_TRI] = sinF.ravel()
        blob[OFF_TRI:NE_TOT] = tri.ravel()
        in_maps.append({"blob": blob})
    return in_maps


def kernel(xs, norm_w, wq, wk, wv, wo):
    from concourse.bass_utils import run_bass_kernel_spmd

    nc = get_nc()
    in_maps = _host_prep(xs, norm_w, wq, wk, wv, wo)
    res = run_bass_kernel_spmd(nc, in_maps, list(range(N_CORES)))
    out = np.concatenate([res.results[c]["out"] for c in range(N_CORES)], axis=1)
    return out.astype(np.float32)



# revision 29
# speedup vs baseline: 1.2288x; 1.0298x over previous
"""Trainium2 Bass kernel for nn_AttentionModule_53223234732422.

Computes: RMSNorm -> QKV projections -> interleaved-pair RoPE on Q,K ->
causal softmax attention (16 heads, head_dim 128) -> output projection.

Sharding (8 NeuronCores, tensor parallel over heads):
  - every core computes the RMSNorm (cheap, avoids an activation collective),
  - each core owns 2 heads: QKV projections with column-sliced weights,
    RoPE, causal attention for those heads,
  - per-head context is AllGathered in bf16 (2 x 0.5 MiB per rank); the
    output projection runs bf16 x bf16 (wo converted on device),
  - output projection is split column-wise: each core produces 256 output
    features from the full gathered context.

Host-side preparation (layout only):
  - ALL inputs are packed into a single flat f32r dram blob per core
    (xsT | wqT | wkT | wvT | woT | cos | sin | tri): each extra PJRT
    operand costs measurable per-execute marshaling time under axon,
  - xs transposed to feature-major [E, S] so contractions land on SBUF
    partitions,
  - norm_w folded into the QKV weights,
  - wq/wk rows permuted per head so RoPE pairs are deinterleaved
    (x0 rows 0..63, x1 rows 64..127); scores are permutation invariant,
  - weights pre-rounded to fp32r (11 mantissa bits, RNE) to match the
    on-device rounding path,
  - cos/sin tables (fp16-arange thetas, like the reference) and the 4
    diagonal causal-mask tiles precomputed.

Dtypes: all matmuls run fp32r (full-rate fp32 path on the PE, 11 mantissa
bits, ~1.2e-4 input rounding; plain fp32 would be 4x slower). PSUM
accumulation is fp32 everywhere. Every tensor an fp32r matmul consumes is
written only by fp32r-typed producers (BIR verifier requirement); weights
are pre-rounded on the host and DMA'd with fp32r-typed endpoints.
Measured on HW: end-to-end relative error 2.4e-3 vs the fp32 reference
(fp32r matmul rounding ~2e-4 + bf16 context/wo ~2e-3; gate is 2e-2).
"""

import sys

sys.path.insert(0, "/opt/trn_rl_repo")

import numpy as np

import concourse.bacc as bacc
import concourse.mybir as mybir
import concourse.tile as tile
from concourse.bass import ds, ts

dt = mybir.dt
AF = mybir.ActivationFunctionType
ALU = mybir.AluOpType

S = 2048
E = 2048
H = 16
D = 128
HALF = D // 2
EPS = 1e-6
THETA = 10000.0
N_CORES = 8
HPC = H // N_CORES  # heads per core
JC = HPC * D  # 256: local q/k/v width
EB = E // N_CORES  # 256: output columns per core
ET = E // 128  # 16 feature tiles
TT = S // 128  # 16 token tiles
NS = S // 512  # 4 token strips
CH = ET // 2  # 8 e-tiles per contraction chunk
INV_SQRT_D = float(1.0 / np.sqrt(np.float32(D)))

F32 = dt.float32
F32R = dt.float32r
BF16 = dt.bfloat16

# Single packed input blob (element offsets, f32/f32r are both 4 bytes).
# One ExternalInput instead of eight: each extra operand costs real
# per-execute marshaling time in the axon PJRT dispatch path.
NE_XS = E * S
NE_W = E * JC  # == E * EB for woT
NE_TRIG = D * S
NE_TRI = 128 * 128
OFF_XS = 0
OFF_WQ = OFF_XS + NE_XS
OFF_WK = OFF_WQ + NE_W
OFF_WV = OFF_WK + NE_W
OFF_WO = OFF_WV + NE_W
OFF_COS = OFF_WO + NE_W
OFF_SIN = OFF_COS + NE_TRIG
OFF_TRI = OFF_SIN + NE_TRIG
NE_TOT = OFF_TRI + NE_TRI

_NC_CACHE = {}


def _build_nc():
    nc = bacc.Bacc(trn_type="TRN2", num_devices=N_CORES)

    blob = nc.dram_tensor("blob", [NE_TOT], F32R, kind="ExternalInput")
    out_ext = nc.dram_tensor("out", [S, EB], F32, kind="ExternalOutput")

    def xs_rows(e):
        return blob[ds(OFF_XS + e * 128 * S, 128 * S)].rearrange(
            "(p t) -> p t", p=128
        )

    def w_chunk(base, chunk):
        return blob[ds(base + chunk * CH * 128 * JC, CH * 128 * JC)].rearrange(
            "(a p j) -> p a j", p=128, j=JC
        )

    def trig(base):
        return (
            blob[ds(base, NE_TRIG)]
            .bitcast(F32)
            .rearrange("(p t) -> p t", p=128)
        )

    rg = [list(range(N_CORES))]

    with tile.TileContext(nc) as tc:
        with (
            tc.tile_pool(name="persist", bufs=1) as pp,
            tc.tile_pool(name="dram", bufs=1, space="DRAM") as dpool,
        ):
            ones_f = pp.tile([128, 1], F32, tag="ones_f")
            ones_r = pp.tile([128, 1], F32R, tag="ones_r")
            ones_b = pp.tile([128, 1], BF16, tag="ones_b")
            eps_sc = pp.tile([1, 1], F32, tag="eps_sc")
            nc.vector.memset(ones_f[:], 1.0)
            nc.vector.tensor_copy(ones_r[:], ones_f[:])
            nc.vector.tensor_copy(ones_b[:], ones_f[:])
            nc.vector.memset(eps_sc[:], EPS)

            # RoPE'd q/k (fp32r; written only by the final rope add) and
            # bf16 token-major v.
            qrope = pp.tile([128, HPC * S], F32R, tag="qrope")
            krope = pp.tile([128, HPC * S], F32R, tag="krope")
            v_sb = pp.tile([128, TT * JC], F32R, tag="v_sb")

            # context is gathered in bf16: halves the AllGather payload and
            # the Phase-E HBM reads; output error stays ~2-4e-3 (gate 2e-2)
            cbounce = [
                dpool.tile([128, S], BF16, tag=f"cb{m}", name=f"cb{m}")
                for m in range(HPC)
            ]
            ag_out = [
                dpool.tile(
                    [N_CORES * 128, S],
                    BF16,
                    addr_space="Shared",
                    tag=f"ag{m}",
                    name=f"ag{m}",
                )
                for m in range(HPC)
            ]

            # ---------- Phase A+C: fused RMS + QKV, xs read ONCE -----------
            # The 1/rms per-token scale commutes out of the e-contraction, so
            # QKV consumes RAW xs (host-pre-rounded to fp32r); the scale is
            # folded into the cos/sin tables (q/k) and a v post-pass.
            # Pass 1 streams e-tiles 8..15, squares them for the running ssq
            # AND contracts them for QKV while resident; pass 2 does the same
            # for tiles 0..7, then computes rms, folds it into the rope
            # tables, ropes q/k, and scales v. This avoids the 8 MiB chunk-1
            # re-read the 2-phase version paid.
            with tc.tile_pool(name="bcC", bufs=1) as bcp:
                bcastR = bcp.tile([128, S], F32, tag="bcastR")
                recip_col = bcp.tile([128, TT], F32, tag="recip_col")
                with (
                    tc.tile_pool(name="xsp", bufs=CH) as xsp,
                    tc.tile_pool(name="rmsp", bufs=1) as rmsp,
                    tc.tile_pool(name="sqp", bufs=2) as sqp,
                    tc.tile_pool(name="wch", bufs=3) as wchp,
                    tc.tile_pool(name="acc", bufs=1) as accp,
                    tc.tile_pool(name="trig", bufs=1) as trigp,
                    tc.tile_pool(name="rsw", bufs=1) as rsp,
                    tc.tile_pool(name="psA", bufs=NS, space="PSUM") as psA,
                    tc.tile_pool(name="psQK", bufs=2, space="PSUM") as psQK,
                    tc.tile_pool(name="psV", bufs=2, space="PSUM") as psV,
                ):
                    rms_row = rmsp.tile([1, S], F32, tag="rms_row")
                    ssq_ps = [
                        psA.tile([1, 512], F32, tag="ssq", name="ssq")
                        for _ in range(NS)
                    ]
                    cos_sb = trigp.tile([D, S], F32, tag="cos_sb")
                    sin_sb = trigp.tile([D, S], F32, tag="sin_sb")
                    nc.sync.dma_start(cos_sb[:], trig(OFF_COS))
                    nc.sync.dma_start(sin_sb[:], trig(OFF_SIN))
                    qacc = accp.tile([128, HPC * S], F32, tag="qacc")
                    kacc = accp.tile([128, HPC * S], F32, tag="kacc")

                    def load_weights(chunk):
                        wtiles = []
                        for wname, wbase in (
                            ("wq", OFF_WQ),
                            ("wk", OFF_WK),
                            ("wv", OFF_WV),
                        ):
                            wc = wchp.tile(
                                [128, CH * JC], F32R, tag="wch",
                                name=f"w{chunk}_{wname}",
                            )
                            nc.sync.dma_start(
                                wc[:].rearrange("p (a j) -> p a j", a=CH),
                                w_chunk(wbase, chunk),
                            )
                            wtiles.append(wc)
                        return wtiles

                    def stream_xs(chunk, first):
                        xh = []
                        for i in range(CH):
                            e = chunk * CH + i
                            xt = xsp.tile([128, S], F32R, tag="xsA", name="xsA")
                            nc.sync.dma_start(xt[:], xs_rows(e))
                            xh.append(xt)
                            for s in range(NS):
                                sq = sqp.tile(
                                    [128, 512], F32R, tag="sq", name="sq"
                                )
                                nc.vector.tensor_mul(
                                    sq[:],
                                    xt[:, ts(s, 512)].bitcast(F32),
                                    xt[:, ts(s, 512)].bitcast(F32),
                                )
                                nc.tensor.matmul(
                                    ssq_ps[s][:],
                                    ones_r[:],
                                    sq[:],
                                    start=(first and i == 0),
                                    stop=((not first) and i == CH - 1),
                                )
                        return xh

                    def qkv_chunk(xh, wq_c, wk_c, wv_c, first):
                        # v projection FIRST -> token-major [t, j], raw sum
                        # (the 1/rms scale is applied in a post-pass):
                        # attention consumes v, so finishing it early lets
                        # head-0 attention start under the q/k tail.
                        for t in range(TT):
                            ps = psV.tile([128, JC], F32, tag="v_ps", name="v_ps")
                            for i in range(CH):
                                nc.tensor.matmul(
                                    ps[:],
                                    xh[i][:, ts(t, 128)],
                                    wv_c[:, ts(i, JC)],
                                    start=(i == 0),
                                    stop=(i == CH - 1),
                                )
                            vsl = v_sb[:, ts(t, JC)]
                            if first:
                                nc.vector.tensor_copy(vsl, ps[:])
                            else:
                                nc.vector.scalar_tensor_tensor(
                                    vsl,
                                    ps[:],
                                    1.0,
                                    vsl.bitcast(F32),
                                    ALU.mult,
                                    ALU.add,
                                )
                        if not first:
                            # apply the per-token 1/rms right away so v is
                            # attention-ready before the rope DVE work queues
                            for t in range(TT):
                                vsl = v_sb[:, ts(t, JC)]
                                nc.vector.tensor_scalar_mul(
                                    vsl, vsl.bitcast(F32), recip_col[:, t : t + 1]
                                )

                        # q and k projections -> d-major [j, t]; head-outer
                        # so head-0's rope completes before any head-1 work
                        # and head-0 attention overlaps the head-1 tail.
                        for m in range(HPC):
                            for wc, acc, rope_dst in (
                                (wq_c, qacc, qrope),
                                (wk_c, kacc, krope),
                            ):
                                for s in range(NS):
                                    ps = psQK.tile(
                                        [128, 512], F32, tag="qk_ps", name="qk_ps"
                                    )
                                    for i in range(CH):
                                        nc.tensor.matmul(
                                            ps[:],
                                            wc[:, ds(i * JC + m * D, D)],
                                            xh[i][:, ts(s, 512)],
                                            start=(i == 0),
                                            stop=(i == CH - 1),
                                        )
                                    asl = acc[:, ds(m * S + s * 512, 512)]
                                    if first:
                                        nc.vector.tensor_copy(asl, ps[:])
                                    else:
                                        nc.vector.scalar_tensor_tensor(
                                            asl, ps[:], 1.0, asl, ALU.mult, ALU.add
                                        )
                                        # RoPE: r = cos*q + sin*swap64(q)
                                        sw = rsp.tile(
                                            [128, 512], F32, tag="rsw", name="rsw"
                                        )
                                        nc.vector.tensor_copy(
                                            sw[0:64, :], asl[64:128, :]
                                        )
                                        nc.vector.tensor_copy(
                                            sw[64:128, :], asl[0:64, :]
                                        )
                                        nc.vector.tensor_mul(
                                            asl, asl, cos_sb[:, ts(s, 512)]
                                        )
                                        nc.vector.tensor_mul(
                                            sw[:], sw[:], sin_sb[:, ts(s, 512)]
                                        )
                                        nc.vector.tensor_tensor(
                                            rope_dst[:, ds(m * S + s * 512, 512)],
                                            asl,
                                            sw[:],
                                            ALU.add,
                                        )

                    # pass 1: e-tiles 8..15 (resident only once)
                    w1 = load_weights(1)
                    xh1 = stream_xs(1, first=True)
                    qkv_chunk(xh1, *w1, first=True)

                    # pass 2: e-tiles 0..7 + rms + rope + v scale
                    w0 = load_weights(0)
                    xh0 = stream_xs(0, first=False)
                    for s in range(NS):
                        # rms = sqrt(ssq/E + eps)
                        nc.scalar.activation(
                            rms_row[0:1, ts(s, 512)],
                            ssq_ps[s][:],
                            AF.Sqrt,
                            bias=eps_sc[0:1, 0:1],
                            scale=1.0 / E,
                        )
                    nc.vector.reciprocal(rms_row[:], rms_row[:])
                    nc.gpsimd.partition_broadcast(bcastR[:], rms_row[0:1, :])
                    # token-major view of the recips for the v scaling;
                    # bounce via DRAM so the strided gather runs on the
                    # DRAM side of the DMA.
                    rrow_d = dpool.tile([1, S], F32, tag="rrow_d", name="rrow_d")
                    nc.sync.dma_start(rrow_d[:], rms_row[:])
                    nc.sync.dma_start(
                        recip_col[:],
                        rrow_d[0, :].rearrange("(a p) -> p a", p=128),
                    )
                    # fold 1/rms into the rope tables (before pass-2 rope)
                    nc.vector.tensor_mul(cos_sb[:], cos_sb[:], bcastR[:])
                    nc.vector.tensor_mul(sin_sb[:], sin_sb[:], bcastR[:])
                    qkv_chunk(xh0, *w0, first=False)

            # ---------------- Phase D: attention ----------------
            with (
                tc.tile_pool(name="attn", bufs=1) as apl,
                tc.tile_pool(name="probs", bufs=8) as prp,
                tc.tile_pool(name="bcD", bufs=2) as bdp,
                tc.tile_pool(name="psS", bufs=4, space="PSUM") as psS,
                tc.tile_pool(name="psCtx", bufs=2, space="PSUM") as psC,
                tc.tile_pool(name="psSum", bufs=2, space="PSUM") as psU,
            ):
                ctx_sb = apl.tile([128, HPC * S], BF16, tag="ctx_sb")
                # single lower-triangle mask tile: tri[i, c] = 1 iff i <= c
                tri = apl.tile([128, 128], F32, tag="tri")
                nc.sync.dma_start(
                    tri[:],
                    blob[ds(OFF_TRI, NE_TRI)]
                    .bitcast(F32)
                    .rearrange("(p c) -> p c", p=128),
                )

                for m in range(HPC):
                    for s in range(NS):
                        n_tk = 4 * (s + 1)
                        ctx_ps = psC.tile([128, 512], F32, tag="ctx_ps", name="ctx_ps")
                        sum_ps = psU.tile([1, 512], F32, tag="sum_ps", name="sum_ps")
                        for j in range(n_tk):
                            p_rel = j - 4 * s
                            # diagonal blocks only attend to tq_local >= off
                            off = 128 * p_rel if p_rel >= 0 else 0
                            n = 512 - off
                            sc = psS.tile([128, 512], F32, tag="sc", name="sc")
                            nc.tensor.matmul(
                                sc[:, 0:n],
                                krope[:, ds(m * S + j * 128, 128)],
                                qrope[:, ds(m * S + s * 512 + off, n)],
                                start=True,
                                stop=True,
                            )
                            pr = prp.tile([128, 512], F32R, tag="probs", name="pr")
                            if p_rel >= 0:
                                # triangle (first 128 cols of the valid range)
                                et = prp.tile([128, 128], F32, tag="expt", name="et")
                                nc.scalar.activation(
                                    et[:], sc[:, 0:128], AF.Exp, scale=INV_SQRT_D
                                )
                                nc.vector.tensor_mul(pr[:, 0:128], et[:], tri[:])
                                if n > 128:
                                    nc.scalar.activation(
                                        pr[:, 128:n],
                                        sc[:, 128:n],
                                        AF.Exp,
                                        scale=INV_SQRT_D,
                                    )
                            else:
                                nc.scalar.activation(
                                    pr[:, 0:n], sc[:, 0:n], AF.Exp, scale=INV_SQRT_D
                                )
                            nc.tensor.matmul(
                                ctx_ps[:, ds(off, n)],
                                v_sb[:, ds(j * JC + m * D, D)],
                                pr[:, 0:n],
                                start=(j == 0),
                                stop=(j == n_tk - 1),
                            )
                            nc.tensor.matmul(
                                sum_ps[0:1, ds(off, n)],
                                ones_r[:],
                                pr[:, 0:n],
                                start=(j == 0),
                                stop=(j == n_tk - 1),
                            )
                        rr = bdp.tile([1, 512], F32, tag="recip", name="rr")
                        nc.vector.reciprocal(rr[:], sum_ps[:])
                        bc = bdp.tile([128, 512], F32, tag="bcD", name="bc")
                        nc.gpsimd.partition_broadcast(bc[:], rr[0:1, :])
                        nc.vector.tensor_mul(
                            ctx_sb[:, ds(m * S + s * 512, 512)], ctx_ps[:], bc[:]
                        )
                        nc.sync.dma_start(
                            cbounce[m][:, ts(s, 512)],
                            ctx_sb[:, ds(m * S + s * 512, 512)],
                        )
                    nc.gpsimd.collective_compute(
                        "AllGather",
                        ALU.bypass,
                        replica_groups=rg,
                        ins=[cbounce[m].opt()],
                        outs=[ag_out[m].opt()],
                    )

            # ---------------- Phase E: output projection ----------------
            # Two passes: the even-head half (ag_out[0]) contracts as soon as
            # the first AllGather lands — overlapping head-1 attention and
            # the second AllGather — with partials parked in SBUF; the odd
            # half then adds on top and evicts.
            with (
                tc.tile_pool(name="ck", bufs=ET) as ckp,
                tc.tile_pool(name="wo", bufs=1) as wop,
                tc.tile_pool(name="ob", bufs=2) as obp,
                tc.tile_pool(name="ob1", bufs=1) as ob1p,
                tc.tile_pool(name="psW", bufs=3, space="PSUM") as psW,
            ):
                woT_r = wop.tile([128, ET * EB], F32R, tag="woT_r")
                nc.sync.dma_start(
                    woT_r[:].rearrange("p (a j) -> p a j", a=ET),
                    blob[ds(OFF_WO, NE_W)].rearrange(
                        "(a p j) -> p a j", p=128, j=EB
                    ),
                )
                woT_sb = wop.tile([128, ET * EB], BF16, tag="woT_sb")
                nc.vector.tensor_copy(woT_sb[:], woT_r[:].bitcast(F32))
                ctxk = []
                for kb in range(ET):
                    ct = ckp.tile([128, S], BF16, tag="ck", name=f"ck{kb}")
                    src = ag_out[0] if kb < CH else ag_out[1]
                    nc.sync.dma_start(ct[:], src[ts(kb % CH, 128), :])
                    ctxk.append(ct)
                ob1 = ob1p.tile([128, TT * EB], F32, tag="ob1")
                for t in range(TT):
                    ps = psW.tile([128, EB], F32, tag="wo_ps", name="wo_ps")
                    for kb in range(CH):
                        nc.tensor.matmul(
                            ps[:],
                            ctxk[kb][:, ts(t, 128)],
                            woT_sb[:, ts(kb, EB)],
                            start=(kb == 0),
                            stop=(kb == CH - 1),
                        )
                    nc.vector.tensor_copy(ob1[:, ts(t, EB)], ps[:])
                for t in range(TT):
                    ps = psW.tile([128, EB], F32, tag="wo_ps", name="wo_ps")
                    for kb in range(CH, ET):
                        nc.tensor.matmul(
                            ps[:],
                            ctxk[kb][:, ts(t, 128)],
                            woT_sb[:, ts(kb, EB)],
                            start=(kb == CH),
                            stop=(kb == ET - 1),
                        )
                    ob = obp.tile([128, EB], F32, tag="ob", name="ob")
                    nc.vector.scalar_tensor_tensor(
                        ob[:], ps[:], 1.0, ob1[:, ts(t, EB)], ALU.mult, ALU.add
                    )
                    nc.sync.dma_start(out_ext[ts(t, 128), :], ob[:])

    nc.compile()
    return nc


def get_nc():
    if "nc" not in _NC_CACHE:
        _NC_CACHE["nc"] = _build_nc()
    return _NC_CACHE["nc"]


def _round_f32r(a):
    """Round fp32 to fp32r (11 explicit mantissa bits) with RNE."""
    u = np.ascontiguousarray(a, dtype=np.float32).view(np.uint32).copy()
    round_bit = (u >> 12) & 1
    u += 0x7FF + round_bit
    u &= np.uint32(0xFFFFF000)
    return u.view(np.float32)


def _rope_tables():
    """thetas with the reference's fp16-arange quirk, then f32 cos/sin."""
    try:
        # Same ops/dtypes as the reference, on the default jax device, so
        # the fp16 pow rounds identically to the reference run in this env.
        import jax.numpy as jnp

        th = (
            THETA ** (-jnp.arange(HALF, dtype=jnp.float16) / HALF)
        ).astype(jnp.float32)
        thetas = np.asarray(th)
    except Exception:
        ar = np.arange(HALF, dtype=np.float16)
        y = -ar / np.float16(HALF)
        thetas = (np.float16(THETA) ** y).astype(np.float32)
    m = np.arange(S, dtype=np.float32)
    ang = m[:, None] * thetas[None, :]  # [S, 64] f32
    cos = np.ascontiguousarray(np.cos(ang).astype(np.float32).T)  # [64, S]
    sin = np.ascontiguousarray(np.sin(ang).astype(np.float32).T)
    cosF = np.concatenate([cos, cos], axis=0)  # [128, S]
    sinF = np.concatenate([-sin, sin], axis=0)
    return np.ascontiguousarray(cosF), np.ascontiguousarray(sinF)


def _host_prep(xs, norm_w, wq, wk, wv, wo):
    xs = np.asarray(xs, dtype=np.float32)
    norm_w = np.asarray(norm_w, dtype=np.float32)
    wq = np.asarray(wq, dtype=np.float32)
    wk = np.asarray(wk, dtype=np.float32)
    wv = np.asarray(wv, dtype=np.float32)
    wo = np.asarray(wo, dtype=np.float32)

    xsT = _round_f32r(np.ascontiguousarray(xs.T))
    cosF, sinF = _rope_tables()

    tri = (
        np.arange(128, dtype=np.int64)[:, None]
        <= np.arange(128, dtype=np.int64)[None, :]
    ).astype(np.float32)

    perm = np.concatenate([np.arange(0, D, 2), np.arange(1, D, 2)])
    wq_n = wq * norm_w[None, :]
    wk_n = wk * norm_w[None, :]
    wv_n = wv * norm_w[None, :]
    f_order = np.concatenate(
        [np.arange(h * D, (h + 1) * D) for h in range(0, H, 2)]
        + [np.arange(h * D, (h + 1) * D) for h in range(1, H, 2)]
    )

    in_maps = []
    for c in range(N_CORES):
        heads = (2 * c, 2 * c + 1)
        rows_qk = np.concatenate([h * D + perm for h in heads])
        rows_v = np.concatenate([np.arange(h * D, (h + 1) * D) for h in heads])
        blob = np.empty(NE_TOT, dtype=np.float32)
        blob[OFF_XS:OFF_WQ] = xsT.ravel()
        blob[OFF_WQ:OFF_WK] = _round_f32r(
            np.ascontiguousarray(wq_n[rows_qk].T)
        ).ravel()
        blob[OFF_WK:OFF_WV] = _round_f32r(
            np.ascontiguousarray(wk_n[rows_qk].T)
        ).ravel()
        blob[OFF_WV:OFF_WO] = _round_f32r(
            np.ascontiguousarray(wv_n[rows_v].T)
        ).ravel()
        blob[OFF_WO:OFF_COS] = _round_f32r(
            np.ascontiguousarray(wo[c * EB : (c + 1) * EB, :].T[f_order, :])
        ).ravel()
        blob[OFF_COS:OFF_SIN] = cosF.ravel()
        blob[OFF_SIN:OFF_TRI] = sinF.ravel()
        blob[OFF_TRI:NE_TOT] = tri.ravel()
        in_maps.append({"blob": blob})
    return in_maps


def kernel(xs, norm_w, wq, wk, wv, wo):
    from concourse.bass_utils import run_bass_kernel_spmd

    nc = get_nc()
    in_maps = _host_prep(xs, norm_w, wq, wk, wv, wo)
    res = run_bass_kernel_spmd(nc, in_maps, list(range(N_CORES)))
    out = np.concatenate([res.results[c]["out"] for c in range(N_CORES)], axis=1)
    return out.astype(np.float32)

